# revision 1
# baseline (speedup 1.0000x reference)
"""Trainium2 Bass kernel for batched dense attention (v5: pair AllGather).

Reference (per batch b):
    q = query @ Wq + bq ; k = key @ Wk + bk ; v = value @ Wv + bv
    out = softmax(BETA * q k^T) v

Sharding: 8 cores = (batch b, seq half h). Core (b,h) computes out rows
[b, h*1024:(h+1)*1024, :].

v5 removes the K-side duplication of v3: each core projects only its OWN
half of the keys (kTr, 65k cycles instead of 131k) and additionally
pre-projects its own half of V' = value @ Wv + bv (65k cycles); the halves
are exchanged within each core pair via DRAM-bounce AllGather
(replica_groups [[0,1],[2,3],[4,5],[6,7]]) overlapped under ~50us of
independent PE work (V'o + q-projection for the kTr gather, phase A for
the V' gather). With V' resident, the attention epilogue is just
out = (P V') * (1/rowsum)  -- v3's phase C (65k cycles) and its PSUM
transposes/copies disappear, and bv rides inside V' exactly
(P(V'+bv)*rrec = PV'*rrec + bv since P@1*rrec == 1).

The program is h-agnostic (SPMD-safe): local results go to a staging tile
-> AllGather input bounce; BOTH halves of kTr/V' are read back from the
gathered output (rank r occupies rows r*128..(r+1)*128), so no core-id
branching is needed.

Per-core PE budget @2.4GHz: kproj-own 65k + V'own 65k + qproj 65k +
scores 131k + PV' 131k ~= 458k cycles ~= 191us (vs v3's 523k ~= 218us).
"""
import ml_dtypes
import numpy as np

import concourse.bass as bass
import concourse.bacc as bacc
import concourse.tile as tile
from concourse import mybir
from concourse.bass_utils import run_bass_kernel_spmd

B, S, D = 4, 2048, 1024
KD = 1024
VD = 1024
BETA = 1.0 / float(np.sqrt(D))
N_CORES = 8
QS = S // 2          # per-core query rows (1024)
KH = S // 2          # own key half (1024)

F32 = mybir.dt.float32
BF16 = mybir.dt.bfloat16

C_D = D // 128       # 8 contraction chunks over D
G_KD = KD // 128     # 8 kd chunks
KT = S // 128        # 16 key tiles (full)
QBLK = 512
NQB = QS // QBLK     # 2 q blocks
NQS = QBLK // 128    # 4 q slices per block

REPLICA_GROUPS = [[2 * i, 2 * i + 1] for i in range(4)]


def build_kernel():
    nc = bacc.Bacc("TRN2", target_bir_lowering=False, debug=False,
                   num_devices=N_CORES)

    qT = nc.dram_tensor("qT", [D, QS], BF16, kind="ExternalInput").ap()
    # kTo/Wk gate the FIRST matmul: host-prearranged 2D SBUF layouts so
    # their DMAs run at full line rate (~2.5us/MB vs ~8.5us/MB for the 3D
    # chunked path). Everything else keeps the v5.1 layout/timing -- the
    # kernel is power-limited and only idle-head trimming is clamp-free.
    kTo = nc.dram_tensor("kTo", [128, 2 * C_D * 512], BF16,
                         kind="ExternalInput").ap()
    vTo = nc.dram_tensor("vTo", [D, KH], BF16, kind="ExternalInput").ap()
    Wq = nc.dram_tensor("Wq", [D, KD], BF16, kind="ExternalInput").ap()
    Wk = nc.dram_tensor("Wk", [128, C_D * KD], BF16,
                        kind="ExternalInput").ap()
    Wv16 = nc.dram_tensor("Wv16", [VD, VD], BF16, kind="ExternalInput").ap()
    bqk = nc.dram_tensor("bqk", [128, 16], F32, kind="ExternalInput").ap()
    bv = nc.dram_tensor("bv", [VD], F32, kind="ExternalInput").ap()
    out = nc.dram_tensor("out", [QS, VD], F32, kind="ExternalOutput").ap()

    with tile.TileContext(nc) as tc:
        _body(tc, qT, kTo, vTo, Wq, Wk, Wv16, bqk, bv, out)
    nc.compile()
    return nc


def _chunked(dram_ap, rows0, nchunk, cols):
    sl = dram_ap[rows0:rows0 + nchunk * 128, 0:cols]
    return sl.rearrange("(c p) x -> p c x", c=nchunk)


def _body(tc, qT, kTo, vTo, Wq, Wk, Wv16, bqk, bv, out):
    nc = tc.nc
    Exp = mybir.ActivationFunctionType.Exp
    mult = mybir.AluOpType.mult
    add = mybir.AluOpType.add

    # ---- persistent constants ------------------------------------------
    const_pool = tc.alloc_tile_pool(name="const", bufs=1)
    constf = const_pool.tile([128, 2210], F32, name="constf")
    bqT = constf[:, 0:8]
    bkT = constf[:, 8:16]
    bvb = constf[:, 16:16 + VD]
    ones_f = constf[:, 1040:1042]
    rrec_all = constf[:, 1042:1058]
    onesrow_f = constf[0:1, 1058:1058 + 128]
    bv_stage = constf[0:1, 1186:1186 + VD]
    onesb = const_pool.tile([128, 2], BF16, name="onesb")

    nc.scalar.dma_start(out=constf[:, 0:16], in_=bqk[:, :])
    nc.scalar.dma_start(out=bv_stage, in_=bv[:])
    nc.vector.memset(ones_f, 1.0)
    nc.vector.memset(onesrow_f, 1.0)
    nc.vector.tensor_copy(onesb[:], ones_f)

    # ---- persistent activations ----------------------------------------
    big_pool = tc.alloc_tile_pool(name="big", bufs=1)
    qTr = big_pool.tile([128, G_KD * QS], BF16, name="qTr")       # 16KB/p
    # kTr2 layout: [128, h(2) x g(8) x 1024]  (rank-major halves)
    kTr2 = big_pool.tile([128, 2 * G_KD * KH], BF16, name="kTr2")  # 32KB/p
    Wv_sb = big_pool.tile([128, C_D * VD], BF16, name="Wv_sb")    # 16KB/p
    pT = big_pool.tile([128, KT * QBLK], BF16, name="pT")         # 16KB/p
    # V'sb layout: [128, kt(16) x 1024] (kt tile = k rows, cols = vd)
    Vp = big_pool.tile([128, KT * VD], BF16, name="Vp")           # 32KB/p
    ost_all = big_pool.tile([128, 2 * 1024], F32, name="ost_all")  # 8KB/p
    ostage = [ost_all[:, i * 1024:(i + 1) * 1024] for i in range(2)]

    # DRAM bounce buffers for the pair AllGathers (one pair per half so
    # each gather dispatches as soon as its half of the projection is done)
    dram_pool = tc.alloc_tile_pool(name="dramp", bufs=1, space="DRAM")
    kAG_in = [dram_pool.tile([128, 4 * 1024], BF16, name=f"kAG_in{i}")
              for i in range(2)]
    kAG_out = [dram_pool.tile([256, 4 * 1024], BF16, name=f"kAG_out{i}")
               for i in range(2)]
    vAG_in = [dram_pool.tile([128, 4 * VD], BF16, name=f"vAG_in{i}")
              for i in range(2)]
    vAG_out = [dram_pool.tile([256, 4 * VD], BF16, name=f"vAG_out{i}")
               for i in range(2)]

    # ---- projection-phase transients -----------------------------------
    proj_pool = tc.alloc_tile_pool(name="proj", bufs=1)
    Wk_sb = proj_pool.tile([128, C_D * KD], BF16, name="Wk_sb")   # 16KB/p
    Wq_sb = proj_pool.tile([128, C_D * KD], BF16, name="Wq_sb")   # 16KB/p
    stg = proj_pool.tile([128, G_KD * 1024], BF16, name="stg")    # 16KB/p

    def xin_tile(engine, src_ap, name):
        t = proj_pool.tile([128, C_D * 512], BF16, name=name, tag="xin",
                           bufs=3)
        engine.dma_start(out=t[:].rearrange("p (c x) -> p c x", c=C_D),
                         in_=src_ap.rearrange("(c p) x -> p c x", c=C_D))
        return t

    # scalar queue: kTo blocks (kproj gate, fast 2D), then vTo, then Wv
    BLKC = C_D * 512
    kin = []
    for b in range(2):
        t = proj_pool.tile([128, BLKC], BF16, name=f"kin{b}", tag="xin",
                           bufs=3)
        nc.scalar.dma_start(out=t[:], in_=kTo[:, b * BLKC:(b + 1) * BLKC])
        kin.append(t)
    vin = [xin_tile(nc.scalar, vTo[:, b * 512:(b + 1) * 512], f"vin{b}")
           for b in range(2)]
    # Wv needed by V'o (~45us in); scalar queue, after the kproj gates.
    # gpsimd stays reserved for the bounce writes + collectives.
    nc.scalar.dma_start(out=Wv_sb[:].rearrange("p (c x) -> p c x", c=C_D),
                        in_=_chunked(Wv16, 0, C_D, VD))
    # sync queue: Wk first (kproj gate, fast 2D), then Wq/qT (consumed
    # later, load during kproj). Gather read-backs ride the scalar queue,
    # which drains its loads by ~25us -- keeping them off sync avoids
    # blocking Wq/qT behind the collectives (in-order FIFO per engine).
    nc.sync.dma_start(out=Wk_sb[:], in_=Wk[:, :])
    nc.sync.dma_start(out=Wq_sb[:].rearrange("p (c x) -> p c x", c=C_D),
                      in_=_chunked(Wq, 0, C_D, KD))
    qin = [xin_tile(nc.sync, qT[:, b * 512:(b + 1) * 512], f"qin{b}")
           for b in range(2)]

    psPro = tc.alloc_tile_pool(name="psPro", bufs=1, space="PSUM")

    # ---- k projection (own half): stg[g,1024] = (Wk^T kTo) + bk --------
    # g-half ordered so each half of stg completes early and its AllGather
    # dispatches while the other half is still computing.
    HG = G_KD // 2
    for gh in range(2):
        for blk in range(2):
            pps = [psPro.tile([128, 512], F32, name=f"kp{gh}_{blk}_{j}",
                              tag="pp", bufs=8) for j in range(HG)]
            for c in range(C_D):
                for j in range(HG):
                    g = gh * HG + j
                    nc.tensor.matmul(
                        pps[j][:],
                        Wk_sb[:, c * KD + g * 128:c * KD + (g + 1) * 128],
                        kin[blk][:, c * 512:(c + 1) * 512],
                        start=(c == 0), stop=(c == C_D - 1))
            for j in range(HG):
                g = gh * HG + j
                nc.vector.tensor_scalar(
                    out=stg[:, g * 1024 + blk * 512:
                            g * 1024 + (blk + 1) * 512],
                    in0=pps[j][:], scalar1=bkT[:, g:g + 1], scalar2=None,
                    op0=add)
        # ship this g-half, gather it, read both ranks' halves back
        h0 = gh * HG * 1024
        nc.gpsimd.dma_start(out=kAG_in[gh][:], in_=stg[:, h0:h0 + HG * 1024])
        nc.gpsimd.collective_compute(
            "AllGather", mybir.AluOpType.bypass,
            replica_groups=REPLICA_GROUPS,
            ins=[kAG_in[gh][:]], outs=[kAG_out[gh][:]])
        for r in range(2):
            nc.scalar.dma_start(
                out=kTr2[:, r * G_KD * KH + h0:r * G_KD * KH + h0 + HG * 1024],
                in_=kAG_out[gh][r * 128:(r + 1) * 128, :])

    # bv broadcast to all partitions via K=1 fp32 matmul
    for n in range(VD // 512):
        bc_ps = psPro.tile([128, 512], F32, name="bc_ps", tag="pp", bufs=8)
        nc.tensor.matmul(bc_ps[:], onesrow_f,
                         bv_stage[:, n * 512:(n + 1) * 512],
                         start=True, stop=True)
        nc.vector.tensor_copy(bvb[:, n * 512:(n + 1) * 512], bc_ps[:])

    # ---- V' (own half): stg[rt,1024] = value_own @ Wv + bv --------------
    # lhsT = vTo chunks [d, k-row slice], rhs = Wv chunks [d, vd cols]
    for blk in range(2):
        pps = [psPro.tile([128, 512], F32, name=f"vp{blk}_{i}", tag="pp",
                          bufs=8) for i in range(G_KD)]
        for c in range(C_D):
            for i in range(G_KD):
                rt, col = divmod(i, 2)
                nc.tensor.matmul(
                    pps[i][:],
                    vin[blk][:, c * 512 + rt * 128:c * 512 + (rt + 1) * 128],
                    Wv_sb[:, c * VD + col * 512:c * VD + (col + 1) * 512],
                    start=(c == 0), stop=(c == C_D - 1))
        for i in range(G_KD):
            rt, col = divmod(i, 2)
            krow = blk * 4 + rt
            nc.vector.tensor_tensor(
                out=stg[:, krow * 1024 + col * 512:krow * 1024 + (col + 1) * 512],
                in0=pps[i][:], in1=bvb[:, col * 512:(col + 1) * 512], op=add)
        # ship this krow-half, gather, read both ranks' quarters back
        b0 = blk * 4 * 1024
        nc.gpsimd.dma_start(out=vAG_in[blk][:], in_=stg[:, b0:b0 + 4 * 1024])
        nc.gpsimd.collective_compute(
            "AllGather", mybir.AluOpType.bypass,
            replica_groups=REPLICA_GROUPS,
            ins=[vAG_in[blk][:]], outs=[vAG_out[blk][:]])
        for r in range(2):
            nc.scalar.dma_start(
                out=Vp[:, r * 8 * VD + b0:r * 8 * VD + b0 + 4 * VD],
                in_=vAG_out[blk][r * 128:(r + 1) * 128, :])

    # ---- q projection: qTr[kd, q] = (Wq^T qT) + bq ----------------------
    for blk in range(NQB):
        pps = [psPro.tile([128, 512], F32, name=f"qp{blk}_{g}", tag="pp",
                          bufs=8) for g in range(G_KD)]
        for c in range(C_D):
            for g in range(G_KD):
                nc.tensor.matmul(
                    pps[g][:],
                    Wq_sb[:, c * KD + g * 128:c * KD + (g + 1) * 128],
                    qin[blk][:, c * 512:(c + 1) * 512],
                    start=(c == 0), stop=(c == C_D - 1))
        for g in range(G_KD):
            nc.vector.tensor_scalar(
                out=qTr[:, g * QS + blk * 512:g * QS + (blk + 1) * 512],
                in0=pps[g][:], scalar1=bqT[:, g:g + 1], scalar2=None, op0=add)

    psPro.release()
    proj_pool.release()

    # ===== main attention loop ==========================================
    # PSUM: sT(2) + rs(1) + acc(4) = 7 banks.
    psM = tc.alloc_tile_pool(name="psM", bufs=1, space="PSUM")
    rs_ps = psM.tile([128, 2 * NQS], F32, name="rs_ps", tag="rs")

    def kslice(g, kt):
        h, kk = divmod(kt, 8)
        base = h * G_KD * KH + g * KH + kk * 128
        return kTr2[:, base:base + 128]

    for qb in range(NQB):
        q0 = qb * QBLK
        # ---- phase A: sT = kTr^T qTr -> exp -> pT ; rowsums on PE ------
        for kt in range(KT):
            sT = psM.tile([128, QBLK], F32, name=f"sT{qb}_{kt}", tag="sT",
                          bufs=2)
            for g in range(G_KD):
                nc.tensor.matmul(
                    sT[:], kslice(g, kt),
                    qTr[:, g * QS + q0:g * QS + q0 + QBLK],
                    start=(g == 0), stop=(g == G_KD - 1))
            nc.scalar.activation(pT[:, kt * QBLK:(kt + 1) * QBLK], sT[:],
                                 Exp, scale=float(BETA))
            for qs in range(NQS):
                nc.tensor.matmul(
                    rs_ps[:, 2 * qs:2 * qs + 2],
                    pT[:, kt * QBLK + qs * 128:kt * QBLK + (qs + 1) * 128],
                    onesb[:],
                    start=(kt == 0 and qs == 0),
                    stop=(kt == KT - 1 and qs == NQS - 1),
                    skip_group_check=True)
        rrec = rrec_all[:, qb * 2 * NQS:(qb + 1) * 2 * NQS]
        nc.vector.reciprocal(rrec, rs_ps[:])

        # ---- phase B': out = (P V') * rrec  (V' already includes bv) ----
        for qs in range(NQS):
            accs = [psM.tile([128, 512], F32, name=f"ob{qb}_{qs}_{p}",
                             tag="acc", bufs=4) for p in range(2)]
            for kt in range(KT):
                for p in range(2):
                    nc.tensor.matmul(
                        accs[p][:],
                        pT[:, kt * QBLK + qs * 128:kt * QBLK + (qs + 1) * 128],
                        Vp[:, kt * VD + p * 512:kt * VD + (p + 1) * 512],
                        start=(kt == 0), stop=(kt == KT - 1))
            ost = ostage[qs % 2]
            for p in range(2):
                nc.vector.tensor_scalar(
                    out=ost[:, p * 512:(p + 1) * 512], in0=accs[p][:],
                    scalar1=rrec[:, 2 * qs:2 * qs + 1], scalar2=None, op0=mult)
            nc.sync.dma_start(
                out=out[q0 + qs * 128:q0 + (qs + 1) * 128, :], in_=ost[:])

    psM.release()
    dram_pool.release()
    big_pool.release()
    const_pool.release()


_NC_CACHE = {}


def _get_nc():
    if "nc" not in _NC_CACHE:
        _NC_CACHE["nc"] = build_kernel()
    return _NC_CACHE["nc"]


def kernel(query, key, value, Wq, bq, Wk, bk, Wv, bv):
    query = np.asarray(query, dtype=np.float32)
    key = np.asarray(key, dtype=np.float32)
    value = np.asarray(value, dtype=np.float32)
    Wq = np.asarray(Wq, dtype=np.float32)
    Wk = np.asarray(Wk, dtype=np.float32)
    Wv = np.asarray(Wv, dtype=np.float32)
    bq = np.asarray(bq, dtype=np.float32)
    bk = np.asarray(bk, dtype=np.float32)
    bv = np.ascontiguousarray(np.asarray(bv, dtype=np.float32))

    nc = _get_nc()
    in_maps = make_in_maps(query, key, value, Wq, bq, Wk, bk, Wv, bv)
    res = run_bass_kernel_spmd(nc, in_maps, list(range(N_CORES)))
    outp = np.empty((B, S, VD), dtype=np.float32)
    for core in range(N_CORES):
        b, h = divmod(core, 2)
        outp[b, h * QS:(h + 1) * QS, :] = res.results[core]["out"]
    return outp


def _arrange_w(W):
    """[D, N] f32 -> bf16 [128, C_D*N], columns (chunk, col)."""
    Dn, N = W.shape
    return np.ascontiguousarray(
        W.astype(ml_dtypes.bfloat16).reshape(C_D, 128, N)
        .transpose(1, 0, 2).reshape(128, C_D * N))


def _arrange_xt(Xt):
    """[D, 1024] f32 (transposed input) -> bf16 [128, 2*C_D*512],
    columns (block, chunk, col)."""
    return np.ascontiguousarray(
        Xt.astype(ml_dtypes.bfloat16).reshape(C_D, 128, 2, 512)
        .transpose(1, 2, 0, 3).reshape(128, 2 * C_D * 512))


def make_in_maps(query, key, value, Wq, bq, Wk, bk, Wv, bv):
    bf16 = ml_dtypes.bfloat16
    Wq16 = Wq.astype(bf16)
    Wk16 = _arrange_w(Wk)
    Wv16 = Wv.astype(bf16)
    bqk = np.ascontiguousarray(
        np.concatenate([bq.reshape(8, 128).T, bk.reshape(8, 128).T], axis=1)
        .astype(np.float32))
    in_maps = []
    for core in range(N_CORES):
        b, h = divmod(core, 2)
        sl = slice(h * KH, (h + 1) * KH)
        in_maps.append({
            "qT": np.ascontiguousarray(query[b, h * QS:(h + 1) * QS, :].T
                                       .astype(bf16)),
            "kTo": _arrange_xt(key[b, sl, :].T),
            "vTo": np.ascontiguousarray(value[b, sl, :].T.astype(bf16)),
            "Wq": Wq16, "Wk": Wk16, "Wv16": Wv16,
            "bqk": bqk, "bv": bv,
        })
    return in_maps



# revision 3
# speedup vs baseline: 1.0172x; 1.0172x over previous
"""Trainium2 Bass kernel for batched dense attention (v6).

Reference (per batch b):
    q = query @ Wq + bq ; k = key @ Wk + bk ; v = value @ Wv + bv
    out = softmax(BETA * q k^T) v

Sharding: 8 cores = (batch b, seq half h). Core (b,h) computes out rows
[b, h*1024:(h+1)*1024, :]. Each core projects its OWN half of K and V';
the pair (2b, 2b+1) exchanges halves via DRAM-bounce AllGather.

v6 changes vs v5 (baseline 275us, PE idle ~64us):
- ALL inputs host-prearranged to flat 2D SBUF layouts -> line-rate DMA
  (~2.5us/MB), spread across scalar/sync/vector queues so the first
  kproj matmul issues at ~6us (was ~20us).
- ONE 2MB AllGather per tensor (k, V') instead of four 1MB ones:
  halves the ~10us/op ncfw fixed cost on the serial cc stream.
- Gather read-backs ride the GPSIMD queue. In v5 they sat on the scalar
  queue ahead of phase A's exp activations (strict in-order FIFO), so
  the exps -- and the PE behind the sT PSUM banks -- stalled until the
  LAST V' gather completed (~155us): ~36us of mid-kernel PE idle plus
  HAM cold-clock oscillation.
- Rowsums fused into phase B with stationary reuse (pT slice is already
  loaded for the two PV matmuls; the N=2 ones-matmul rides along) --
  drops v5's interleaved rowsum matmuls + their LDWEIGHTS in phase A.
- ~4us PE warmup burst so projections run at 2.4GHz from the start
  (HAM un-throttles after ~3.4us of sustained PE activity).

Per-core PE budget @2.4GHz: warmup 4 + kproj-own 27 + V'own 27 +
qproj 27 + scores 55 + PV 55 + rowsum ride-along ~6 = ~200us.
"""
import ml_dtypes
import numpy as np

import concourse.bass as bass
import concourse.bacc as bacc
import concourse.tile as tile
from concourse import mybir
from concourse.bass_utils import run_bass_kernel_spmd

B, S, D = 4, 2048, 1024
KD = 1024
VD = 1024
BETA = 1.0 / float(np.sqrt(D))
N_CORES = 8
QS = S // 2          # per-core query rows (1024)
KH = S // 2          # own key half (1024)

F32 = mybir.dt.float32
BF16 = mybir.dt.bfloat16

C_D = D // 128       # 8 contraction chunks over D
G_KD = KD // 128     # 8 kd chunks
KT = S // 128        # 16 key tiles (full)
QBLK = 512
NQB = QS // QBLK     # 2 q blocks
NQS = QBLK // 128    # 4 q slices per block
BLKC = C_D * 512     # 4096 cols per input block
HCOL = G_KD * KH     # 8192 cols per rank in kTr2 / Vp

REPLICA_GROUPS = [[2 * i, 2 * i + 1] for i in range(4)]


def build_kernel():
    nc = bacc.Bacc("TRN2", target_bir_lowering=False, debug=False,
                   num_devices=N_CORES)

    # All inputs host-prearranged into [128, ...] row layouts so every
    # load is a flat 2D DMA at line rate.
    qT2 = nc.dram_tensor("qT2", [128, 2 * BLKC], BF16,
                         kind="ExternalInput").ap()
    kTo = nc.dram_tensor("kTo", [128, 2 * BLKC], BF16,
                         kind="ExternalInput").ap()
    vTo2 = nc.dram_tensor("vTo2", [128, 2 * BLKC], BF16,
                          kind="ExternalInput").ap()
    Wq2 = nc.dram_tensor("Wq2", [128, C_D * KD], BF16,
                         kind="ExternalInput").ap()
    Wk = nc.dram_tensor("Wk", [128, C_D * KD], BF16,
                        kind="ExternalInput").ap()
    Wv2 = nc.dram_tensor("Wv2", [128, C_D * VD], BF16,
                         kind="ExternalInput").ap()
    bqk = nc.dram_tensor("bqk", [128, 16], F32, kind="ExternalInput").ap()
    bv = nc.dram_tensor("bv", [VD], F32, kind="ExternalInput").ap()
    out = nc.dram_tensor("out", [QS, VD], F32, kind="ExternalOutput").ap()

    with tile.TileContext(nc) as tc:
        _body(tc, qT2, kTo, vTo2, Wq2, Wk, Wv2, bqk, bv, out)
    nc.compile()
    return nc


def _body(tc, qT2, kTo, vTo2, Wq2, Wk, Wv2, bqk, bv, out):
    nc = tc.nc
    Exp = mybir.ActivationFunctionType.Exp
    mult = mybir.AluOpType.mult
    add = mybir.AluOpType.add

    # ---- persistent constants ------------------------------------------
    const_pool = tc.alloc_tile_pool(name="const", bufs=1)
    constf = const_pool.tile([128, 2210], F32, name="constf")
    bqT = constf[:, 0:8]
    bkT = constf[:, 8:16]
    bvb = constf[:, 16:16 + VD]
    ones_f = constf[:, 1040:1042]
    rrec_all = constf[:, 1042:1058]
    onesrow_f = constf[0:1, 1058:1058 + 128]
    bv_stage = constf[0:1, 1186:1186 + VD]
    onesb = const_pool.tile([128, 2], BF16, name="onesb")
    warm = const_pool.tile([128, 128], BF16, name="warm")

    nc.scalar.dma_start(out=constf[:, 0:16], in_=bqk[:, :])
    nc.scalar.dma_start(out=bv_stage, in_=bv[:])
    nc.vector.memset(warm[:], 0.0)
    nc.vector.memset(ones_f, 1.0)
    nc.vector.memset(onesrow_f, 1.0)
    nc.vector.tensor_copy(onesb[:], ones_f)

    # ---- persistent activations ----------------------------------------
    big_pool = tc.alloc_tile_pool(name="big", bufs=1)
    qTr = big_pool.tile([128, G_KD * QS], BF16, name="qTr")       # 16KB/p
    # kTr2 layout: [128, rank(2) x g(8) x 1024]  (rank-major halves)
    kTr2 = big_pool.tile([128, 2 * HCOL], BF16, name="kTr2")      # 32KB/p
    # Vp layout: [128, kt(16) x 1024] (kt tile = k rows, cols = vd)
    Vp = big_pool.tile([128, KT * VD], BF16, name="Vp")           # 32KB/p

    # DRAM bounce buffers for the two pair AllGathers
    dram_pool = tc.alloc_tile_pool(name="dramp", bufs=1, space="DRAM")
    kAG_in = dram_pool.tile([128, HCOL], BF16, name="kAG_in")
    kAG_out = dram_pool.tile([256, HCOL], BF16, name="kAG_out")
    vAG_in = dram_pool.tile([128, HCOL], BF16, name="vAG_in")
    vAG_out = dram_pool.tile([256, HCOL], BF16, name="vAG_out")

    # ---- projection-phase transients -----------------------------------
    proj_pool = tc.alloc_tile_pool(name="proj", bufs=1)
    Wk_sb = proj_pool.tile([128, C_D * KD], BF16, name="Wk_sb")   # 16KB/p
    Wq_sb = proj_pool.tile([128, C_D * KD], BF16, name="Wq_sb")   # 16KB/p
    Wv_sb = proj_pool.tile([128, C_D * VD], BF16, name="Wv_sb")   # 16KB/p
    stg = proj_pool.tile([128, HCOL], BF16, name="stg")           # 16KB/p
    kin = [proj_pool.tile([128, BLKC], BF16, name=f"kin{b}")
           for b in range(2)]
    vin = [proj_pool.tile([128, BLKC], BF16, name=f"vin{b}")
           for b in range(2)]
    qin = [proj_pool.tile([128, BLKC], BF16, name=f"qin{b}")
           for b in range(2)]

    # Input loads, spread across queues; kproj gates (kin, Wk) first on
    # their queue so the PE starts ~6us in.
    for b in range(2):
        nc.scalar.dma_start(out=kin[b][:],
                            in_=kTo[:, b * BLKC:(b + 1) * BLKC])
    nc.sync.dma_start(out=Wk_sb[:], in_=Wk[:, :])
    nc.sync.dma_start(out=Wq_sb[:], in_=Wq2[:, :])
    for b in range(2):
        nc.sync.dma_start(out=qin[b][:],
                          in_=qT2[:, b * BLKC:(b + 1) * BLKC])
    nc.gpsimd.dma_start(out=Wv_sb[:], in_=Wv2[:, :])
    for b in range(2):
        nc.gpsimd.dma_start(out=vin[b][:],
                            in_=vTo2[:, b * BLKC:(b + 1) * BLKC])

    psPro = tc.alloc_tile_pool(name="psPro", bufs=1, space="PSUM")

    # ---- PE warmup: ~4us of dummy matmuls so HAM un-throttles before
    # the real work (two accumulation groups stream with no inter-MM sync)
    for wg in range(2):
        wps = psPro.tile([128, 128], F32, name=f"warmps{wg}", tag="pp",
                         bufs=8)
        for i in range(20):
            nc.tensor.matmul(wps[:], warm[:], warm[:],
                             start=(i == 0), stop=(i == 19))

    # ---- k projection (own half): stg[g,1024] = (Wk^T kTo) + bk --------
    HG = G_KD // 2
    for gh in range(2):
        for blk in range(2):
            pps = [psPro.tile([128, 512], F32, name=f"kp{gh}_{blk}_{j}",
                              tag="pp", bufs=8) for j in range(HG)]
            for c in range(C_D):
                for j in range(HG):
                    g = gh * HG + j
                    nc.tensor.matmul(
                        pps[j][:],
                        Wk_sb[:, c * KD + g * 128:c * KD + (g + 1) * 128],
                        kin[blk][:, c * 512:(c + 1) * 512],
                        start=(c == 0), stop=(c == C_D - 1))
            for j in range(HG):
                g = gh * HG + j
                nc.vector.tensor_scalar(
                    out=stg[:, g * 1024 + blk * 512:
                            g * 1024 + (blk + 1) * 512],
                    in0=pps[j][:], scalar1=bkT[:, g:g + 1], scalar2=None,
                    op0=add)
        # bounce this g-half to the gather input as soon as it's done
        h0 = gh * HG * 1024
        nc.gpsimd.dma_start(out=kAG_in[:, h0:h0 + HG * 1024],
                            in_=stg[:, h0:h0 + HG * 1024])
    # one 2MB-in pair AllGather for k; read BOTH rank halves back
    nc.gpsimd.collective_compute(
        "AllGather", mybir.AluOpType.bypass,
        replica_groups=REPLICA_GROUPS,
        ins=[kAG_in[:]], outs=[kAG_out[:]])
    for r in range(2):
        nc.gpsimd.dma_start(
            out=kTr2[:, r * HCOL:(r + 1) * HCOL],
            in_=kAG_out[r * 128:(r + 1) * 128, :])

    # bv broadcast to all partitions via K=1 fp32 matmul
    for n in range(VD // 512):
        bc_ps = psPro.tile([128, 512], F32, name="bc_ps", tag="pp", bufs=8)
        nc.tensor.matmul(bc_ps[:], onesrow_f,
                         bv_stage[:, n * 512:(n + 1) * 512],
                         start=True, stop=True)
        nc.vector.tensor_copy(bvb[:, n * 512:(n + 1) * 512], bc_ps[:])

    # ---- V' (own half): stg[krow,1024] = value_own @ Wv + bv ------------
    for blk in range(2):
        pps = [psPro.tile([128, 512], F32, name=f"vp{blk}_{i}", tag="pp",
                          bufs=8) for i in range(G_KD)]
        for c in range(C_D):
            for i in range(G_KD):
                rt, col = divmod(i, 2)
                nc.tensor.matmul(
                    pps[i][:],
                    vin[blk][:, c * 512 + rt * 128:c * 512 + (rt + 1) * 128],
                    Wv_sb[:, c * VD + col * 512:c * VD + (col + 1) * 512],
                    start=(c == 0), stop=(c == C_D - 1))
        for i in range(G_KD):
            rt, col = divmod(i, 2)
            krow = blk * 4 + rt
            nc.vector.tensor_tensor(
                out=stg[:, krow * 1024 + col * 512:
                        krow * 1024 + (col + 1) * 512],
                in0=pps[i][:], in1=bvb[:, col * 512:(col + 1) * 512], op=add)
        b0 = blk * 4 * 1024
        nc.gpsimd.dma_start(out=vAG_in[:, b0:b0 + 4 * 1024],
                            in_=stg[:, b0:b0 + 4 * 1024])
    nc.gpsimd.collective_compute(
        "AllGather", mybir.AluOpType.bypass,
        replica_groups=REPLICA_GROUPS,
        ins=[vAG_in[:]], outs=[vAG_out[:]])
    for r in range(2):
        nc.gpsimd.dma_start(
            out=Vp[:, r * HCOL:(r + 1) * HCOL],
            in_=vAG_out[r * 128:(r + 1) * 128, :])

    # ---- q projection: qTr[kd, q] = (Wq^T qT) + bq ----------------------
    for blk in range(NQB):
        pps = [psPro.tile([128, 512], F32, name=f"qp{blk}_{g}", tag="pp",
                          bufs=8) for g in range(G_KD)]
        for c in range(C_D):
            for g in range(G_KD):
                nc.tensor.matmul(
                    pps[g][:],
                    Wq_sb[:, c * KD + g * 128:c * KD + (g + 1) * 128],
                    qin[blk][:, c * 512:(c + 1) * 512],
                    start=(c == 0), stop=(c == C_D - 1))
        for g in range(G_KD):
            nc.vector.tensor_scalar(
                out=qTr[:, g * QS + blk * 512:g * QS + (blk + 1) * 512],
                in0=pps[g][:], scalar1=bqT[:, g:g + 1], scalar2=None, op0=add)

    psPro.release()
    proj_pool.release()

    # ---- main-loop transients (reuse the projection space) --------------
    main_pool = tc.alloc_tile_pool(name="main", bufs=1)
    pT = main_pool.tile([128, KT * QBLK], BF16, name="pT")        # 16KB/p
    ost_all = main_pool.tile([128, 2 * 1024], F32, name="ost_all")  # 8KB/p
    ostage = [ost_all[:, i * 1024:(i + 1) * 1024] for i in range(2)]

    # ===== main attention loop ==========================================
    # PSUM: sT(2) + rs(1) + acc(4) = 7 banks.
    psM = tc.alloc_tile_pool(name="psM", bufs=1, space="PSUM")
    rs_ps = psM.tile([128, 2 * NQS], F32, name="rs_ps", tag="rs")

    def kslice(g, kt):
        h, kk = divmod(kt, 8)
        base = h * HCOL + g * KH + kk * 128
        return kTr2[:, base:base + 128]

    for qb in range(NQB):
        q0 = qb * QBLK
        # ---- phase A: sT = kTr^T qTr -> exp -> pT ----------------------
        for kt in range(KT):
            sT = psM.tile([128, QBLK], F32, name=f"sT{qb}_{kt}", tag="sT",
                          bufs=2)
            for g in range(G_KD):
                nc.tensor.matmul(
                    sT[:], kslice(g, kt),
                    qTr[:, g * QS + q0:g * QS + q0 + QBLK],
                    start=(g == 0), stop=(g == G_KD - 1))
            nc.scalar.activation(pT[:, kt * QBLK:(kt + 1) * QBLK], sT[:],
                                 Exp, scale=float(BETA))

        # ---- phase B: out = (P V') * (1/rowsum); rowsum rides along ----
        for qs in range(NQS):
            accs = [psM.tile([128, 512], F32, name=f"ob{qb}_{qs}_{p}",
                             tag="acc", bufs=4) for p in range(2)]
            for kt in range(KT):
                lhs = pT[:, kt * QBLK + qs * 128:kt * QBLK + (qs + 1) * 128]
                for p in range(2):
                    nc.tensor.matmul(
                        accs[p][:], lhs,
                        Vp[:, kt * VD + p * 512:kt * VD + (p + 1) * 512],
                        start=(kt == 0), stop=(kt == KT - 1))
                nc.tensor.matmul(
                    rs_ps[:, 2 * qs:2 * qs + 2], lhs, onesb[:],
                    start=(kt == 0), stop=(kt == KT - 1),
                    skip_group_check=True)
            rrec = rrec_all[:, (qb * NQS + qs) * 2:(qb * NQS + qs) * 2 + 2]
            nc.vector.reciprocal(rrec, rs_ps[:, 2 * qs:2 * qs + 2])
            ost = ostage[qs % 2]
            for p in range(2):
                nc.vector.tensor_scalar(
                    out=ost[:, p * 512:(p + 1) * 512], in0=accs[p][:],
                    scalar1=rrec[:, 0:1], scalar2=None, op0=mult)
            nc.sync.dma_start(
                out=out[q0 + qs * 128:q0 + (qs + 1) * 128, :], in_=ost[:])

    psM.release()
    main_pool.release()
    dram_pool.release()
    big_pool.release()
    const_pool.release()


_NC_CACHE = {}


def _get_nc():
    if "nc" not in _NC_CACHE:
        _NC_CACHE["nc"] = build_kernel()
    return _NC_CACHE["nc"]


def kernel(query, key, value, Wq, bq, Wk, bk, Wv, bv):
    query = np.asarray(query, dtype=np.float32)
    key = np.asarray(key, dtype=np.float32)
    value = np.asarray(value, dtype=np.float32)
    Wq = np.asarray(Wq, dtype=np.float32)
    Wk = np.asarray(Wk, dtype=np.float32)
    Wv = np.asarray(Wv, dtype=np.float32)
    bq = np.asarray(bq, dtype=np.float32)
    bk = np.asarray(bk, dtype=np.float32)
    bv = np.ascontiguousarray(np.asarray(bv, dtype=np.float32))

    nc = _get_nc()
    in_maps = make_in_maps(query, key, value, Wq, bq, Wk, bk, Wv, bv)
    res = run_bass_kernel_spmd(nc, in_maps, list(range(N_CORES)))
    outp = np.empty((B, S, VD), dtype=np.float32)
    for core in range(N_CORES):
        b, h = divmod(core, 2)
        outp[b, h * QS:(h + 1) * QS, :] = res.results[core]["out"]
    return outp


def _arrange_w(W):
    """[D, N] f32 -> bf16 [128, C_D*N], columns (chunk, col)."""
    Dn, N = W.shape
    return np.ascontiguousarray(
        W.astype(ml_dtypes.bfloat16).reshape(C_D, 128, N)
        .transpose(1, 0, 2).reshape(128, C_D * N))


def _arrange_xt(Xt):
    """[D, 1024] f32 (transposed input) -> bf16 [128, 2*C_D*512],
    columns (block, chunk, col)."""
    return np.ascontiguousarray(
        Xt.astype(ml_dtypes.bfloat16).reshape(C_D, 128, 2, 512)
        .transpose(1, 2, 0, 3).reshape(128, 2 * C_D * 512))


def make_in_maps(query, key, value, Wq, bq, Wk, bk, Wv, bv):
    Wq16 = _arrange_w(Wq)
    Wk16 = _arrange_w(Wk)
    Wv16 = _arrange_w(Wv)
    bqk = np.ascontiguousarray(
        np.concatenate([bq.reshape(8, 128).T, bk.reshape(8, 128).T], axis=1)
        .astype(np.float32))
    in_maps = []
    for core in range(N_CORES):
        b, h = divmod(core, 2)
        sl = slice(h * KH, (h + 1) * KH)
        in_maps.append({
            "qT2": _arrange_xt(query[b, h * QS:(h + 1) * QS, :].T),
            "kTo": _arrange_xt(key[b, sl, :].T),
            "vTo2": _arrange_xt(value[b, sl, :].T),
            "Wq2": Wq16, "Wk": Wk16, "Wv2": Wv16,
            "bqk": bqk, "bv": bv,
        })
    return in_maps


# revision 6
# speedup vs baseline: 1.0937x; 1.0752x over previous
"""Trainium2 Bass kernel for batched dense attention (v6).

Reference (per batch b):
    q = query @ Wq + bq ; k = key @ Wk + bk ; v = value @ Wv + bv
    out = softmax(BETA * q k^T) v

Sharding: 8 cores = (batch b, seq half h). Core (b,h) computes out rows
[b, h*1024:(h+1)*1024, :]. Each core projects its OWN half of K and V';
the pair (2b, 2b+1) exchanges halves via DRAM-bounce AllGather.

v6 changes vs v5 (baseline 275us, PE idle ~64us):
- ALL inputs host-prearranged to flat 2D SBUF layouts -> line-rate DMA
  (~2.5us/MB), spread across scalar/sync/vector queues so the first
  kproj matmul issues at ~6us (was ~20us).
- ONE 2MB AllGather per tensor (k, V') instead of four 1MB ones:
  halves the ~10us/op ncfw fixed cost on the serial cc stream.
- Gather read-backs ride the GPSIMD queue. In v5 they sat on the scalar
  queue ahead of phase A's exp activations (strict in-order FIFO), so
  the exps -- and the PE behind the sT PSUM banks -- stalled until the
  LAST V' gather completed (~155us): ~36us of mid-kernel PE idle plus
  HAM cold-clock oscillation.
- Rowsums fused into phase B with stationary reuse (pT slice is already
  loaded for the two PV matmuls; the N=2 ones-matmul rides along) --
  drops v5's interleaved rowsum matmuls + their LDWEIGHTS in phase A.
- ~4us PE warmup burst so projections run at 2.4GHz from the start
  (HAM un-throttles after ~3.4us of sustained PE activity).

Per-core PE budget @2.4GHz: warmup 4 + kproj-own 27 + V'own 27 +
qproj 27 + scores 55 + PV 55 + rowsum ride-along ~6 = ~200us.
"""
import ml_dtypes
import numpy as np

import concourse.bass as bass
import concourse.bacc as bacc
import concourse.tile as tile
from concourse import mybir
from concourse.bass_utils import run_bass_kernel_spmd

B, S, D = 4, 2048, 1024
KD = 1024
VD = 1024
BETA = 1.0 / float(np.sqrt(D))
N_CORES = 8
QS = S // 2          # per-core query rows (1024)
KH = S // 2          # own key half (1024)

F32 = mybir.dt.float32
BF16 = mybir.dt.bfloat16

C_D = D // 128       # 8 contraction chunks over D
G_KD = KD // 128     # 8 kd chunks
KT = S // 128        # 16 key tiles (full)
QBLK = 512
NQB = QS // QBLK     # 2 q blocks
NQS = QBLK // 128    # 4 q slices per block
BLKC = C_D * 512     # 4096 cols per input block
HCOL = G_KD * KH     # 8192 cols per rank in kTr2 / Vp

REPLICA_GROUPS = [[2 * i, 2 * i + 1] for i in range(4)]


def build_kernel():
    nc = bacc.Bacc("TRN2", target_bir_lowering=False, debug=False,
                   num_devices=N_CORES)

    # All inputs host-prearranged into [128, ...] row layouts so every
    # load is a flat 2D DMA at line rate.
    qT2 = nc.dram_tensor("qT2", [128, 2 * BLKC], BF16,
                         kind="ExternalInput").ap()
    kTo = nc.dram_tensor("kTo", [128, 2 * BLKC], BF16,
                         kind="ExternalInput").ap()
    vTo2 = nc.dram_tensor("vTo2", [128, 2 * BLKC], BF16,
                          kind="ExternalInput").ap()
    Wq2 = nc.dram_tensor("Wq2", [128, C_D * KD], BF16,
                         kind="ExternalInput").ap()
    Wk = nc.dram_tensor("Wk", [128, C_D * KD], BF16,
                        kind="ExternalInput").ap()
    Wv2 = nc.dram_tensor("Wv2", [128, C_D * VD], BF16,
                         kind="ExternalInput").ap()
    bqk = nc.dram_tensor("bqk", [128, 16], F32, kind="ExternalInput").ap()
    bv = nc.dram_tensor("bv", [VD], F32, kind="ExternalInput").ap()
    out = nc.dram_tensor("out", [QS, VD], F32, kind="ExternalOutput").ap()

    with tile.TileContext(nc) as tc:
        _body(tc, qT2, kTo, vTo2, Wq2, Wk, Wv2, bqk, bv, out)
    nc.compile()
    return nc


def _body(tc, qT2, kTo, vTo2, Wq2, Wk, Wv2, bqk, bv, out):
    nc = tc.nc
    Exp = mybir.ActivationFunctionType.Exp
    mult = mybir.AluOpType.mult
    add = mybir.AluOpType.add

    # ---- persistent constants ------------------------------------------
    const_pool = tc.alloc_tile_pool(name="const", bufs=1)
    constf = const_pool.tile([128, 2210], F32, name="constf")
    bqT = constf[:, 0:8]
    bkT = constf[:, 8:16]
    bvb = constf[:, 16:16 + VD]
    ones_f = constf[:, 1040:1042]
    rrec_all = constf[:, 1042:1058]
    onesrow_f = constf[0:1, 1058:1058 + 128]
    bv_stage = constf[0:1, 1186:1186 + VD]
    onesb = const_pool.tile([128, 2], BF16, name="onesb")
    warm = const_pool.tile([128, 128], BF16, name="warm")

    nc.scalar.dma_start(out=constf[:, 0:16], in_=bqk[:, :])
    nc.scalar.dma_start(out=bv_stage, in_=bv[:])
    nc.vector.memset(warm[:], 0.0)
    nc.vector.memset(ones_f, 1.0)
    nc.vector.memset(onesrow_f, 1.0)
    nc.vector.tensor_copy(onesb[:], ones_f)

    # ---- persistent activations ----------------------------------------
    big_pool = tc.alloc_tile_pool(name="big", bufs=1)
    qTr = big_pool.tile([128, G_KD * QS], BF16, name="qTr")       # 16KB/p
    # kTr2 layout: [128, rank(2) x g(8) x 1024]  (rank-major halves)
    kTr2 = big_pool.tile([128, 2 * HCOL], BF16, name="kTr2")      # 32KB/p
    # Vp layout: [128, kt(16) x 1024] (kt tile = k rows, cols = vd)
    Vp = big_pool.tile([128, KT * VD], BF16, name="Vp")           # 32KB/p

    # DRAM bounce buffers for the two pair AllGathers
    dram_pool = tc.alloc_tile_pool(name="dramp", bufs=1, space="DRAM")
    kAG_in = dram_pool.tile([128, HCOL], BF16, name="kAG_in")
    kAG_out = dram_pool.tile([256, HCOL], BF16, name="kAG_out")
    vAG_in = dram_pool.tile([128, HCOL], BF16, name="vAG_in")
    vAG_out = dram_pool.tile([256, HCOL], BF16, name="vAG_out")

    # ---- projection-phase transients -----------------------------------
    proj_pool = tc.alloc_tile_pool(name="proj", bufs=1)
    Wk_sb = proj_pool.tile([128, C_D * KD], BF16, name="Wk_sb")   # 16KB/p
    Wq_sb = proj_pool.tile([128, C_D * KD], BF16, name="Wq_sb")   # 16KB/p
    Wv_sb = proj_pool.tile([128, C_D * VD], BF16, name="Wv_sb")   # 16KB/p
    stg = proj_pool.tile([128, HCOL], BF16, name="stg")           # 16KB/p
    kin = [proj_pool.tile([128, BLKC], BF16, name=f"kin{b}")
           for b in range(2)]
    vin = [proj_pool.tile([128, BLKC], BF16, name=f"vin{b}")
           for b in range(2)]
    qin = [proj_pool.tile([128, BLKC], BF16, name=f"qin{b}")
           for b in range(2)]

    # Input loads: ONLY the two hardware-DGE queues (scalar, sync) carry
    # data; gpsimd (software DGE, slow descgen) keeps just the collective
    # triggers. DMA bandwidth is one shared ~360GB/s pool, so order by
    # deadline: kproj gates (kin, Wk) first, V' gates next, qproj last.
    for b in range(2):
        nc.scalar.dma_start(out=kin[b][:],
                            in_=kTo[:, b * BLKC:(b + 1) * BLKC])
    nc.sync.dma_start(out=Wk_sb[:], in_=Wk[:, :])
    for b in range(2):
        nc.sync.dma_start(out=vin[b][:],
                          in_=vTo2[:, b * BLKC:(b + 1) * BLKC])
    nc.sync.dma_start(out=Wv_sb[:], in_=Wv2[:, :])
    nc.sync.dma_start(out=Wq_sb[:], in_=Wq2[:, :])
    for b in range(2):
        nc.sync.dma_start(out=qin[b][:],
                          in_=qT2[:, b * BLKC:(b + 1) * BLKC])

    psPro = tc.alloc_tile_pool(name="psPro", bufs=1, space="PSUM")

    # ---- PE warmup: ~4us of dummy matmuls so HAM un-throttles before
    # the real work (two accumulation groups stream with no inter-MM sync)
    for wg in range(2):
        wps = psPro.tile([128, 128], F32, name=f"warmps{wg}", tag="pp",
                         bufs=8)
        for i in range(20):
            nc.tensor.matmul(wps[:], warm[:], warm[:],
                             start=(i == 0), stop=(i == 19))

    # ---- k projection (own half): stg[g,1024] = (Wk^T kTo) + bk --------
    HG = G_KD // 2
    for gh in range(2):
        for blk in range(2):
            pps = [psPro.tile([128, 512], F32, name=f"kp{gh}_{blk}_{j}",
                              tag="pp", bufs=8) for j in range(HG)]
            for c in range(C_D):
                for j in range(HG):
                    g = gh * HG + j
                    nc.tensor.matmul(
                        pps[j][:],
                        Wk_sb[:, c * KD + g * 128:c * KD + (g + 1) * 128],
                        kin[blk][:, c * 512:(c + 1) * 512],
                        start=(c == 0), stop=(c == C_D - 1))
            for j in range(HG):
                g = gh * HG + j
                nc.vector.tensor_scalar(
                    out=stg[:, g * 1024 + blk * 512:
                            g * 1024 + (blk + 1) * 512],
                    in0=pps[j][:], scalar1=bkT[:, g:g + 1], scalar2=None,
                    op0=add)
        # bounce this g-half to the gather input as soon as it's done
        h0 = gh * HG * 1024
        nc.scalar.dma_start(out=kAG_in[:, h0:h0 + HG * 1024],
                            in_=stg[:, h0:h0 + HG * 1024])
    # one 2MB-in pair AllGather for k (trigger-only on gpsimd)
    nc.gpsimd.collective_compute(
        "AllGather", mybir.AluOpType.bypass,
        replica_groups=REPLICA_GROUPS,
        ins=[kAG_in[:]], outs=[kAG_out[:]])

    # bv broadcast to all partitions via K=1 fp32 matmul
    for n in range(VD // 512):
        bc_ps = psPro.tile([128, 512], F32, name="bc_ps", tag="pp", bufs=8)
        nc.tensor.matmul(bc_ps[:], onesrow_f,
                         bv_stage[:, n * 512:(n + 1) * 512],
                         start=True, stop=True)
        nc.vector.tensor_copy(bvb[:, n * 512:(n + 1) * 512], bc_ps[:])

    # ---- V' (own half): stg[krow,1024] = value_own @ Wv + bv ------------
    for blk in range(2):
        pps = [psPro.tile([128, 512], F32, name=f"vp{blk}_{i}", tag="pp",
                          bufs=8) for i in range(G_KD)]
        for c in range(C_D):
            for i in range(G_KD):
                rt, col = divmod(i, 2)
                nc.tensor.matmul(
                    pps[i][:],
                    vin[blk][:, c * 512 + rt * 128:c * 512 + (rt + 1) * 128],
                    Wv_sb[:, c * VD + col * 512:c * VD + (col + 1) * 512],
                    start=(c == 0), stop=(c == C_D - 1))
        for i in range(G_KD):
            rt, col = divmod(i, 2)
            krow = blk * 4 + rt
            nc.vector.tensor_tensor(
                out=stg[:, krow * 1024 + col * 512:
                        krow * 1024 + (col + 1) * 512],
                in0=pps[i][:], in1=bvb[:, col * 512:(col + 1) * 512], op=add)
        b0 = blk * 4 * 1024
        nc.scalar.dma_start(out=vAG_in[:, b0:b0 + 4 * 1024],
                            in_=stg[:, b0:b0 + 4 * 1024])
    nc.gpsimd.collective_compute(
        "AllGather", mybir.AluOpType.bypass,
        replica_groups=REPLICA_GROUPS,
        ins=[vAG_in[:]], outs=[vAG_out[:]])
    # read-backs: k halves on scalar (retire before phase A's exps),
    # V' halves on sync (retire before the out writes)
    for r in range(2):
        nc.scalar.dma_start(
            out=kTr2[:, r * HCOL:(r + 1) * HCOL],
            in_=kAG_out[r * 128:(r + 1) * 128, :])
    for r in range(2):
        nc.sync.dma_start(
            out=Vp[:, r * HCOL:(r + 1) * HCOL],
            in_=vAG_out[r * 128:(r + 1) * 128, :])

    # ---- q projection: qTr[kd, q] = (Wq^T qT) + bq ----------------------
    for blk in range(NQB):
        pps = [psPro.tile([128, 512], F32, name=f"qp{blk}_{g}", tag="pp",
                          bufs=8) for g in range(G_KD)]
        for c in range(C_D):
            for g in range(G_KD):
                nc.tensor.matmul(
                    pps[g][:],
                    Wq_sb[:, c * KD + g * 128:c * KD + (g + 1) * 128],
                    qin[blk][:, c * 512:(c + 1) * 512],
                    start=(c == 0), stop=(c == C_D - 1))
        for g in range(G_KD):
            nc.vector.tensor_scalar(
                out=qTr[:, g * QS + blk * 512:g * QS + (blk + 1) * 512],
                in0=pps[g][:], scalar1=bqT[:, g:g + 1], scalar2=None, op0=add)

    psPro.release()
    proj_pool.release()

    # ---- main-loop transients (reuse the projection space) --------------
    main_pool = tc.alloc_tile_pool(name="main", bufs=1)
    pT = main_pool.tile([128, KT * QBLK], BF16, name="pT")        # 16KB/p
    ost_all = main_pool.tile([128, 2 * 1024], F32, name="ost_all")  # 8KB/p
    ostage = [ost_all[:, i * 1024:(i + 1) * 1024] for i in range(2)]

    # ===== main attention loop ==========================================
    # PSUM: sT(2) + rs(1) + acc(4) = 7 banks.
    psM = tc.alloc_tile_pool(name="psM", bufs=1, space="PSUM")
    rs_ps = psM.tile([128, 2 * NQS], F32, name="rs_ps", tag="rs")

    def kslice(g, kt):
        h, kk = divmod(kt, 8)
        base = h * HCOL + g * KH + kk * 128
        return kTr2[:, base:base + 128]

    for qb in range(NQB):
        q0 = qb * QBLK
        # ---- phase A: sT = kTr^T qTr -> exp -> pT ----------------------
        for kt in range(KT):
            sT = psM.tile([128, QBLK], F32, name=f"sT{qb}_{kt}", tag="sT",
                          bufs=2)
            for g in range(G_KD):
                nc.tensor.matmul(
                    sT[:], kslice(g, kt),
                    qTr[:, g * QS + q0:g * QS + q0 + QBLK],
                    start=(g == 0), stop=(g == G_KD - 1))
            nc.scalar.activation(pT[:, kt * QBLK:(kt + 1) * QBLK], sT[:],
                                 Exp, scale=float(BETA))

        # ---- phase B: out = (P V') * (1/rowsum); rowsum rides along ----
        for qs in range(NQS):
            accs = [psM.tile([128, 512], F32, name=f"ob{qb}_{qs}_{p}",
                             tag="acc", bufs=4) for p in range(2)]
            for kt in range(KT):
                lhs = pT[:, kt * QBLK + qs * 128:kt * QBLK + (qs + 1) * 128]
                for p in range(2):
                    nc.tensor.matmul(
                        accs[p][:], lhs,
                        Vp[:, kt * VD + p * 512:kt * VD + (p + 1) * 512],
                        start=(kt == 0), stop=(kt == KT - 1))
                nc.tensor.matmul(
                    rs_ps[:, 2 * qs:2 * qs + 2], lhs, onesb[:],
                    start=(kt == 0), stop=(kt == KT - 1),
                    skip_group_check=True)
            rrec = rrec_all[:, (qb * NQS + qs) * 2:(qb * NQS + qs) * 2 + 2]
            nc.vector.reciprocal(rrec, rs_ps[:, 2 * qs:2 * qs + 2])
            ost = ostage[qs % 2]
            for p in range(2):
                nc.vector.tensor_scalar(
                    out=ost[:, p * 512:(p + 1) * 512], in0=accs[p][:],
                    scalar1=rrec[:, 0:1], scalar2=None, op0=mult)
            nc.sync.dma_start(
                out=out[q0 + qs * 128:q0 + (qs + 1) * 128, :], in_=ost[:])

    psM.release()
    main_pool.release()
    dram_pool.release()
    big_pool.release()
    const_pool.release()


_NC_CACHE = {}


def _get_nc():
    if "nc" not in _NC_CACHE:
        _NC_CACHE["nc"] = build_kernel()
    return _NC_CACHE["nc"]


def kernel(query, key, value, Wq, bq, Wk, bk, Wv, bv):
    query = np.asarray(query, dtype=np.float32)
    key = np.asarray(key, dtype=np.float32)
    value = np.asarray(value, dtype=np.float32)
    Wq = np.asarray(Wq, dtype=np.float32)
    Wk = np.asarray(Wk, dtype=np.float32)
    Wv = np.asarray(Wv, dtype=np.float32)
    bq = np.asarray(bq, dtype=np.float32)
    bk = np.asarray(bk, dtype=np.float32)
    bv = np.ascontiguousarray(np.asarray(bv, dtype=np.float32))

    nc = _get_nc()
    in_maps = make_in_maps(query, key, value, Wq, bq, Wk, bk, Wv, bv)
    res = run_bass_kernel_spmd(nc, in_maps, list(range(N_CORES)))
    outp = np.empty((B, S, VD), dtype=np.float32)
    for core in range(N_CORES):
        b, h = divmod(core, 2)
        outp[b, h * QS:(h + 1) * QS, :] = res.results[core]["out"]
    return outp


def _arrange_w(W):
    """[D, N] f32 -> bf16 [128, C_D*N], columns (chunk, col)."""
    Dn, N = W.shape
    return np.ascontiguousarray(
        W.astype(ml_dtypes.bfloat16).reshape(C_D, 128, N)
        .transpose(1, 0, 2).reshape(128, C_D * N))


def _arrange_xt(Xt):
    """[D, 1024] f32 (transposed input) -> bf16 [128, 2*C_D*512],
    columns (block, chunk, col)."""
    return np.ascontiguousarray(
        Xt.astype(ml_dtypes.bfloat16).reshape(C_D, 128, 2, 512)
        .transpose(1, 2, 0, 3).reshape(128, 2 * C_D * 512))


def make_in_maps(query, key, value, Wq, bq, Wk, bk, Wv, bv):
    Wq16 = _arrange_w(Wq)
    Wk16 = _arrange_w(Wk)
    Wv16 = _arrange_w(Wv)
    bqk = np.ascontiguousarray(
        np.concatenate([bq.reshape(8, 128).T, bk.reshape(8, 128).T], axis=1)
        .astype(np.float32))
    in_maps = []
    for core in range(N_CORES):
        b, h = divmod(core, 2)
        sl = slice(h * KH, (h + 1) * KH)
        in_maps.append({
            "qT2": _arrange_xt(query[b, h * QS:(h + 1) * QS, :].T),
            "kTo": _arrange_xt(key[b, sl, :].T),
            "vTo2": _arrange_xt(value[b, sl, :].T),
            "Wq2": Wq16, "Wk": Wk16, "Wv2": Wv16,
            "bqk": bqk, "bv": bv,
        })
    return in_maps


# revision 10
# speedup vs baseline: 1.1186x; 1.0228x over previous
"""Trainium2 Bass kernel for batched dense attention (v6).

Reference (per batch b):
    q = query @ Wq + bq ; k = key @ Wk + bk ; v = value @ Wv + bv
    out = softmax(BETA * q k^T) v

Sharding: 8 cores = (batch b, seq half h). Core (b,h) computes out rows
[b, h*1024:(h+1)*1024, :]. Each core projects its OWN half of K and V';
the pair (2b, 2b+1) exchanges halves via DRAM-bounce AllGather.

v6 changes vs v5 (baseline 275us, PE idle ~64us):
- ALL inputs host-prearranged to flat 2D SBUF layouts -> line-rate DMA
  (~2.5us/MB), spread across scalar/sync/vector queues so the first
  kproj matmul issues at ~6us (was ~20us).
- ONE 2MB AllGather per tensor (k, V') instead of four 1MB ones:
  halves the ~10us/op ncfw fixed cost on the serial cc stream.
- Gather read-backs ride the GPSIMD queue. In v5 they sat on the scalar
  queue ahead of phase A's exp activations (strict in-order FIFO), so
  the exps -- and the PE behind the sT PSUM banks -- stalled until the
  LAST V' gather completed (~155us): ~36us of mid-kernel PE idle plus
  HAM cold-clock oscillation.
- Rowsums fused into phase B with stationary reuse (pT slice is already
  loaded for the two PV matmuls; the N=2 ones-matmul rides along) --
  drops v5's interleaved rowsum matmuls + their LDWEIGHTS in phase A.
- ~4us PE warmup burst so projections run at 2.4GHz from the start
  (HAM un-throttles after ~3.4us of sustained PE activity).

Per-core PE budget @2.4GHz: warmup 4 + kproj-own 27 + V'own 27 +
qproj 27 + scores 55 + PV 55 + rowsum ride-along ~6 = ~200us.
"""
import ml_dtypes
import numpy as np

import concourse.bass as bass
import concourse.bacc as bacc
import concourse.tile as tile
from concourse import mybir
from concourse.bass_utils import run_bass_kernel_spmd

B, S, D = 4, 2048, 1024
KD = 1024
VD = 1024
BETA = 1.0 / float(np.sqrt(D))
N_CORES = 8
QS = S // 2          # per-core query rows (1024)
KH = S // 2          # own key half (1024)

F32 = mybir.dt.float32
BF16 = mybir.dt.bfloat16

C_D = D // 128       # 8 contraction chunks over D
G_KD = KD // 128     # 8 kd chunks
KT = S // 128        # 16 key tiles (full)
QBLK = 512
NQB = QS // QBLK     # 2 q blocks
NQS = QBLK // 128    # 4 q slices per block
BLKC = C_D * 512     # 4096 cols per input block
HCOL = G_KD * KH     # 8192 cols per rank in kTr2 / Vp

REPLICA_GROUPS = [[2 * i, 2 * i + 1] for i in range(4)]


def build_kernel():
    nc = bacc.Bacc("TRN2", target_bir_lowering=False, debug=False,
                   num_devices=N_CORES)

    # All inputs host-prearranged into [128, ...] row layouts so every
    # load is a flat 2D DMA at line rate.
    qT2 = nc.dram_tensor("qT2", [128, 2 * BLKC], BF16,
                         kind="ExternalInput").ap()
    kTo = nc.dram_tensor("kTo", [128, 2 * BLKC], BF16,
                         kind="ExternalInput").ap()
    vTo2 = nc.dram_tensor("vTo2", [128, 2 * BLKC], BF16,
                          kind="ExternalInput").ap()
    Wq2 = nc.dram_tensor("Wq2", [128, C_D * KD], BF16,
                         kind="ExternalInput").ap()
    Wk = nc.dram_tensor("Wk", [128, C_D * KD], BF16,
                        kind="ExternalInput").ap()
    Wv2 = nc.dram_tensor("Wv2", [128, C_D * VD], BF16,
                         kind="ExternalInput").ap()
    bqk = nc.dram_tensor("bqk", [128, 16], F32, kind="ExternalInput").ap()
    bv = nc.dram_tensor("bv", [VD], F32, kind="ExternalInput").ap()
    out = nc.dram_tensor("out", [QS, VD], F32, kind="ExternalOutput").ap()

    with tile.TileContext(nc) as tc:
        _body(tc, qT2, kTo, vTo2, Wq2, Wk, Wv2, bqk, bv, out)
    nc.compile()
    return nc


def _body(tc, qT2, kTo, vTo2, Wq2, Wk, Wv2, bqk, bv, out):
    nc = tc.nc
    Exp = mybir.ActivationFunctionType.Exp
    mult = mybir.AluOpType.mult
    add = mybir.AluOpType.add

    # ---- persistent constants ------------------------------------------
    const_pool = tc.alloc_tile_pool(name="const", bufs=1)
    constf = const_pool.tile([128, 2210], F32, name="constf")
    bqT = constf[:, 0:8]
    bkT = constf[:, 8:16]
    bvb = constf[:, 16:16 + VD]
    ones_f = constf[:, 1040:1042]
    rrec_all = constf[:, 1042:1058]
    onesrow_f = constf[0:1, 1058:1058 + 128]
    bv_stage = constf[0:1, 1186:1186 + VD]
    onesb = const_pool.tile([128, 2], BF16, name="onesb")
    warm = const_pool.tile([128, 128], BF16, name="warm")

    nc.scalar.dma_start(out=constf[:, 0:16], in_=bqk[:, :])
    nc.scalar.dma_start(out=bv_stage, in_=bv[:])
    nc.vector.memset(warm[:], 0.0)
    nc.vector.memset(ones_f, 1.0)
    nc.vector.memset(onesrow_f, 1.0)
    nc.vector.tensor_copy(onesb[:], ones_f)

    # ---- persistent activations ----------------------------------------
    big_pool = tc.alloc_tile_pool(name="big", bufs=1)
    qTr = big_pool.tile([128, G_KD * QS], BF16, name="qTr")       # 16KB/p
    # kTr2 layout: [128, rank(2) x g(8) x 1024]  (rank-major halves)
    kTr2 = big_pool.tile([128, 2 * HCOL], BF16, name="kTr2")      # 32KB/p
    # Vp layout: [128, kt(16) x 1024] (kt tile = k rows, cols = vd)
    Vp = big_pool.tile([128, KT * VD], BF16, name="Vp")           # 32KB/p

    # DRAM bounce buffers for the two pair AllGathers
    dram_pool = tc.alloc_tile_pool(name="dramp", bufs=1, space="DRAM")
    kAG_in = dram_pool.tile([128, HCOL], BF16, name="kAG_in")
    kAG_out = dram_pool.tile([256, HCOL], BF16, name="kAG_out")
    vAG_in = dram_pool.tile([128, HCOL], BF16, name="vAG_in")
    vAG_out = dram_pool.tile([256, HCOL], BF16, name="vAG_out")

    # ---- projection-phase transients -----------------------------------
    proj_pool = tc.alloc_tile_pool(name="proj", bufs=1)
    Wk_sb = proj_pool.tile([128, C_D * KD], BF16, name="Wk_sb")   # 16KB/p
    Wq_sb = proj_pool.tile([128, C_D * KD], BF16, name="Wq_sb")   # 16KB/p
    Wv_sb = proj_pool.tile([128, C_D * VD], BF16, name="Wv_sb")   # 16KB/p
    stg = proj_pool.tile([128, HCOL], BF16, name="stg")           # 16KB/p
    kin = [proj_pool.tile([128, BLKC], BF16, name=f"kin{b}")
           for b in range(2)]
    vin = [proj_pool.tile([128, BLKC], BF16, name=f"vin{b}")
           for b in range(2)]
    qin = [proj_pool.tile([128, BLKC], BF16, name=f"qin{b}")
           for b in range(2)]

    # Input loads: ONLY the two hardware-DGE queues (scalar, sync) carry
    # data; gpsimd (software DGE, slow descgen) keeps just the collective
    # triggers. The DMA system has a ~8us dead start, the sync queue comes
    # alive first, and aggregate bandwidth is ~350GB/s -- so the kproj
    # gates (Wk, kin) ride sync in interleaved c-chunk pieces (subtile
    # deps let each matmul start as soon as its chunk lands), everything
    # else follows by deadline.
    for j in range(4):
        nc.sync.dma_start(out=Wk_sb[:, j * 2 * KD:(j + 1) * 2 * KD],
                          in_=Wk[:, j * 2 * KD:(j + 1) * 2 * KD])
        nc.sync.dma_start(out=kin[0][:, j * 1024:(j + 1) * 1024],
                          in_=kTo[:, j * 1024:(j + 1) * 1024])
    for j in range(4):
        nc.sync.dma_start(out=kin[1][:, j * 1024:(j + 1) * 1024],
                          in_=kTo[:, BLKC + j * 1024:BLKC + (j + 1) * 1024])
    for b in range(2):
        nc.scalar.dma_start(out=vin[b][:],
                            in_=vTo2[:, b * BLKC:(b + 1) * BLKC])
    nc.scalar.dma_start(out=Wv_sb[:], in_=Wv2[:, :])
    nc.sync.dma_start(out=Wq_sb[:], in_=Wq2[:, :])
    for b in range(2):
        nc.sync.dma_start(out=qin[b][:],
                          in_=qT2[:, b * BLKC:(b + 1) * BLKC])

    psPro = tc.alloc_tile_pool(name="psPro", bufs=1, space="PSUM")

    # ---- PE warmup: ~6us of dummy matmuls so HAM un-throttles before
    # the real work (accumulation groups stream with no inter-MM sync)
    for wg in range(3):
        wps = psPro.tile([128, 128], F32, name=f"warmps{wg}", tag="pp",
                         bufs=8)
        for i in range(19):
            nc.tensor.matmul(wps[:], warm[:], warm[:],
                             start=(i == 0), stop=(i == 18))

    # ---- k projection (own half): stg[g,1024] = (Wk^T kTo) + bk --------
    HG = G_KD // 2
    for gh in range(2):
        for blk in range(2):
            pps = [psPro.tile([128, 512], F32, name=f"kp{gh}_{blk}_{j}",
                              tag="pp", bufs=8) for j in range(HG)]
            for c in range(C_D):
                for j in range(HG):
                    g = gh * HG + j
                    nc.tensor.matmul(
                        pps[j][:],
                        Wk_sb[:, c * KD + g * 128:c * KD + (g + 1) * 128],
                        kin[blk][:, c * 512:(c + 1) * 512],
                        start=(c == 0), stop=(c == C_D - 1))
            for j in range(HG):
                g = gh * HG + j
                nc.vector.tensor_scalar(
                    out=stg[:, g * 1024 + blk * 512:
                            g * 1024 + (blk + 1) * 512],
                    in0=pps[j][:], scalar1=bkT[:, g:g + 1], scalar2=None,
                    op0=add)
        # bounce this g-half to the gather input as soon as it's done
        h0 = gh * HG * 1024
        nc.scalar.dma_start(out=kAG_in[:, h0:h0 + HG * 1024],
                            in_=stg[:, h0:h0 + HG * 1024])
    # one 2MB-in pair AllGather for k (trigger-only on gpsimd)
    nc.gpsimd.collective_compute(
        "AllGather", mybir.AluOpType.bypass,
        replica_groups=REPLICA_GROUPS,
        ins=[kAG_in[:]], outs=[kAG_out[:]])

    # bv broadcast to all partitions via K=1 fp32 matmul
    for n in range(VD // 512):
        bc_ps = psPro.tile([128, 512], F32, name="bc_ps", tag="pp", bufs=8)
        nc.tensor.matmul(bc_ps[:], onesrow_f,
                         bv_stage[:, n * 512:(n + 1) * 512],
                         start=True, stop=True)
        nc.vector.tensor_copy(bvb[:, n * 512:(n + 1) * 512], bc_ps[:])

    # ---- V' (own half): stg[krow,1024] = value_own @ Wv + bv ------------
    for blk in range(2):
        pps = [psPro.tile([128, 512], F32, name=f"vp{blk}_{i}", tag="pp",
                          bufs=8) for i in range(G_KD)]
        for c in range(C_D):
            for i in range(G_KD):
                rt, col = divmod(i, 2)
                nc.tensor.matmul(
                    pps[i][:],
                    vin[blk][:, c * 512 + rt * 128:c * 512 + (rt + 1) * 128],
                    Wv_sb[:, c * VD + col * 512:c * VD + (col + 1) * 512],
                    start=(c == 0), stop=(c == C_D - 1))
        for i in range(G_KD):
            rt, col = divmod(i, 2)
            krow = blk * 4 + rt
            nc.vector.tensor_tensor(
                out=stg[:, krow * 1024 + col * 512:
                        krow * 1024 + (col + 1) * 512],
                in0=pps[i][:], in1=bvb[:, col * 512:(col + 1) * 512], op=add)
        b0 = blk * 4 * 1024
        nc.sync.dma_start(out=vAG_in[:, b0:b0 + 4 * 1024],
                          in_=stg[:, b0:b0 + 4 * 1024])
    nc.gpsimd.collective_compute(
        "AllGather", mybir.AluOpType.bypass,
        replica_groups=REPLICA_GROUPS,
        ins=[vAG_in[:]], outs=[vAG_out[:]])
    # read-backs: k halves on scalar (retire before phase A's exps),
    # V' halves on sync (retire before the out writes). Each queue's
    # FIFO stays monotone in semaphore-wait time.
    for r in range(2):
        nc.scalar.dma_start(
            out=kTr2[:, r * HCOL:(r + 1) * HCOL],
            in_=kAG_out[r * 128:(r + 1) * 128, :])
    for r in range(2):
        nc.sync.dma_start(
            out=Vp[:, r * HCOL:(r + 1) * HCOL],
            in_=vAG_out[r * 128:(r + 1) * 128, :])

    # ---- q projection: qTr[kd, q] = (Wq^T qT) + bq ----------------------
    for blk in range(NQB):
        pps = [psPro.tile([128, 512], F32, name=f"qp{blk}_{g}", tag="pp",
                          bufs=8) for g in range(G_KD)]
        for c in range(C_D):
            for g in range(G_KD):
                nc.tensor.matmul(
                    pps[g][:],
                    Wq_sb[:, c * KD + g * 128:c * KD + (g + 1) * 128],
                    qin[blk][:, c * 512:(c + 1) * 512],
                    start=(c == 0), stop=(c == C_D - 1))
        for g in range(G_KD):
            nc.vector.tensor_scalar(
                out=qTr[:, g * QS + blk * 512:g * QS + (blk + 1) * 512],
                in0=pps[g][:], scalar1=bqT[:, g:g + 1], scalar2=None, op0=add)

    psPro.release()
    proj_pool.release()

    # ---- main-loop transients (reuse the projection space) --------------
    main_pool = tc.alloc_tile_pool(name="main", bufs=1)
    pT = main_pool.tile([128, NQB * KT * QBLK], BF16, name="pT")  # 32KB/p
    ost_all = main_pool.tile([128, 2 * 1024], F32, name="ost_all")  # 8KB/p
    ostage = [ost_all[:, i * 1024:(i + 1) * 1024] for i in range(2)]

    # ===== main attention loop ==========================================
    # Order: A(qb0), A(qb1), B(qb0), B(qb1) -- phase B starts a full
    # 27us later than in A/B/A/B order, buying the V' exchange that much
    # slack against collective latency + cross-core launch skew.
    # PSUM: sT(2) + rs(1) + acc(4) = 7 banks.
    psM = tc.alloc_tile_pool(name="psM", bufs=1, space="PSUM")
    rs_ps = psM.tile([128, 2 * NQS], F32, name="rs_ps", tag="rs")

    def kslice(g, kt):
        h, kk = divmod(kt, 8)
        base = h * HCOL + g * KH + kk * 128
        return kTr2[:, base:base + 128]

    def pTq(qb):
        return pT[:, qb * KT * QBLK:(qb + 1) * KT * QBLK]

    # ---- phase A: sT = kTr^T qTr -> exp -> pT --------------------------
    for qb in range(NQB):
        q0 = qb * QBLK
        for kt in range(KT):
            sT = psM.tile([128, QBLK], F32, name=f"sT{qb}_{kt}", tag="sT",
                          bufs=2)
            for g in range(G_KD):
                nc.tensor.matmul(
                    sT[:], kslice(g, kt),
                    qTr[:, g * QS + q0:g * QS + q0 + QBLK],
                    start=(g == 0), stop=(g == G_KD - 1))
            nc.scalar.activation(
                pTq(qb)[:, kt * QBLK:(kt + 1) * QBLK], sT[:],
                Exp, scale=float(BETA))

    # ---- phase B: out = (P V') * (1/rowsum); rowsum rides along --------
    for qb in range(NQB):
        q0 = qb * QBLK
        for qs in range(NQS):
            accs = [psM.tile([128, 512], F32, name=f"ob{qb}_{qs}_{p}",
                             tag="acc", bufs=4) for p in range(2)]
            for kt in range(KT):
                lhs = pTq(qb)[:, kt * QBLK + qs * 128:
                              kt * QBLK + (qs + 1) * 128]
                for p in range(2):
                    nc.tensor.matmul(
                        accs[p][:], lhs,
                        Vp[:, kt * VD + p * 512:kt * VD + (p + 1) * 512],
                        start=(kt == 0), stop=(kt == KT - 1))
                nc.tensor.matmul(
                    rs_ps[:, 2 * qs:2 * qs + 2], lhs, onesb[:],
                    start=(kt == 0), stop=(kt == KT - 1),
                    skip_group_check=True)
            rrec = rrec_all[:, (qb * NQS + qs) * 2:(qb * NQS + qs) * 2 + 2]
            nc.vector.reciprocal(rrec, rs_ps[:, 2 * qs:2 * qs + 2])
            ost = ostage[qs % 2]
            for p in range(2):
                nc.vector.tensor_scalar(
                    out=ost[:, p * 512:(p + 1) * 512], in0=accs[p][:],
                    scalar1=rrec[:, 0:1], scalar2=None, op0=mult)
            nc.sync.dma_start(
                out=out[q0 + qs * 128:q0 + (qs + 1) * 128, :], in_=ost[:])

    psM.release()
    main_pool.release()
    dram_pool.release()
    big_pool.release()
    const_pool.release()


_NC_CACHE = {}


def _get_nc():
    if "nc" not in _NC_CACHE:
        _NC_CACHE["nc"] = build_kernel()
    return _NC_CACHE["nc"]


def kernel(query, key, value, Wq, bq, Wk, bk, Wv, bv):
    query = np.asarray(query, dtype=np.float32)
    key = np.asarray(key, dtype=np.float32)
    value = np.asarray(value, dtype=np.float32)
    Wq = np.asarray(Wq, dtype=np.float32)
    Wk = np.asarray(Wk, dtype=np.float32)
    Wv = np.asarray(Wv, dtype=np.float32)
    bq = np.asarray(bq, dtype=np.float32)
    bk = np.asarray(bk, dtype=np.float32)
    bv = np.ascontiguousarray(np.asarray(bv, dtype=np.float32))

    nc = _get_nc()
    in_maps = make_in_maps(query, key, value, Wq, bq, Wk, bk, Wv, bv)
    res = run_bass_kernel_spmd(nc, in_maps, list(range(N_CORES)))
    outp = np.empty((B, S, VD), dtype=np.float32)
    for core in range(N_CORES):
        b, h = divmod(core, 2)
        outp[b, h * QS:(h + 1) * QS, :] = res.results[core]["out"]
    return outp


def _arrange_w(W):
    """[D, N] f32 -> bf16 [128, C_D*N], columns (chunk, col)."""
    Dn, N = W.shape
    return np.ascontiguousarray(
        W.astype(ml_dtypes.bfloat16).reshape(C_D, 128, N)
        .transpose(1, 0, 2).reshape(128, C_D * N))


def _arrange_xt(Xt):
    """[D, 1024] f32 (transposed input) -> bf16 [128, 2*C_D*512],
    columns (block, chunk, col)."""
    return np.ascontiguousarray(
        Xt.astype(ml_dtypes.bfloat16).reshape(C_D, 128, 2, 512)
        .transpose(1, 2, 0, 3).reshape(128, 2 * C_D * 512))


def make_in_maps(query, key, value, Wq, bq, Wk, bk, Wv, bv):
    Wq16 = _arrange_w(Wq)
    Wk16 = _arrange_w(Wk)
    Wv16 = _arrange_w(Wv)
    bqk = np.ascontiguousarray(
        np.concatenate([bq.reshape(8, 128).T, bk.reshape(8, 128).T], axis=1)
        .astype(np.float32))
    in_maps = []
    for core in range(N_CORES):
        b, h = divmod(core, 2)
        sl = slice(h * KH, (h + 1) * KH)
        in_maps.append({
            "qT2": _arrange_xt(query[b, h * QS:(h + 1) * QS, :].T),
            "kTo": _arrange_xt(key[b, sl, :].T),
            "vTo2": _arrange_xt(value[b, sl, :].T),
            "Wq2": Wq16, "Wk": Wk16, "Wv2": Wv16,
            "bqk": bqk, "bv": bv,
        })
    return in_maps
